# revision 1
# baseline (speedup 1.0000x reference)
"""Trainium2 Bass kernel for the MACE-style symmetric contraction.

Math (per node b, feature c, with emb = node_embeddings[b, c, :] (16,)):
    w{3,2,1}[k, c] = sum_e attr[b, e] * W{3,2,1}[e, k, c]
    out3[x, y] = sum_{i,k} emb[i] * w3[k] * U3[0, x, y, i, k]        (16, 16)
    M3[x, y]   = out3[x, y] + sum_k2 U2[0, x, y, k2] * w2[k2]
    o2[x]      = sum_y M3[x, y] * emb[y] + U1[0, x, 0] * w1[0]
    o1         = sum_x o2[x] * emb[x]
    output[b, c] = o1

Mapping: columns = (node-in-tile, c) pairs, 4 nodes x 128 c = 512 cols/tile.
The (i, k) contraction (k-major, 368 rows + 4 U2 rows) runs on the PE as
3 accumulating matmuls per output half (x,y) -> 256 rows in 2 halves of 128.
The y- and x-contractions with emb are elementwise multiplies (DVE) plus
selection/ones matmuls (PE). All PE operands are f16; accumulation is fp32.
"""

import os

import numpy as np

# ---------------- problem constants (hardcoded per contract) ----------------
N, C, Y, E = 3000, 128, 16, 10
Z3, Z2, Z1 = 23, 4, 1
NCORES = 8
NB = 376                # nodes per core (3008 = 8*376, padded)
NPAD = NCORES * NB
TB = 4                  # nodes per tile
F = TB * C              # 512 columns per tile
NT = NB // TB           # 94 tiles
KK = Z3 + Z2 + Z1       # 28 packed k rows in wflat
WROW = KK * C           # 3584: wflat row length
KM = (128, 128, 116)    # contraction chunk K sizes (368 ik rows + 4 U2 rows)

_CACHE = {}


def _build_program(nb):
    """Build the single-core Bass program (SPMD: same program, all cores)."""
    import concourse.bass as bass
    import concourse.mybir as mybir
    import concourse.tile as tile
    from concourse import bacc

    f16, f32 = mybir.dt.float16, mybir.dt.float32
    nt = nb // TB
    nc = bacc.Bacc(None, target_bir_lowering=False)

    embT_d = nc.dram_tensor("embT", [Y, nb * C], f16, kind="ExternalInput")
    attrT_d = nc.dram_tensor("attrT", [E, nb], f16, kind="ExternalInput")
    wcat_d = nc.dram_tensor("wcat", [E, WROW], f16, kind="ExternalInput")
    u3s_d = nc.dram_tensor("u3s", [2, 3, 128, 128], f16, kind="ExternalInput")
    sel_d = nc.dram_tensor("sel", [2, 128, 16], f16, kind="ExternalInput")
    onesu1_d = nc.dram_tensor("onesu1", [48, 1], f16, kind="ExternalInput")
    out_d = nc.dram_tensor("out", [nb, C], f32, kind="ExternalOutput")

    with tile.TileContext(nc) as tc:
        with tc.tile_pool(name="consts", bufs=1) as consts, \
             tc.tile_pool(name="dram", bufs=1, space="DRAM") as dpool:
            # stationaries, loaded once
            u3s = []
            for h in range(2):
                row = []
                for m in range(3):
                    t = consts.tile([128, 128], f16, tag=f"u3s{h}{m}")
                    nc.sync.dma_start(out=t[:], in_=u3s_d[h, m])
                    row.append(t)
                u3s.append(row)
            sel = []
            for h in range(2):
                t = consts.tile([128, 16], f16, tag=f"sel{h}")
                nc.sync.dma_start(out=t[:], in_=sel_d[h])
                sel.append(t)
            onesu1 = consts.tile([48, 1], f16, tag="onesu1")
            nc.sync.dma_start(out=onesu1[:], in_=onesu1_d[:])

            # PE warm-up: ~30 dependency-free matmuls (~8 us) push the HAM
            # activity window to K=8/8 (2.4 GHz) before real work starts;
            # the steady pipeline never idles long enough to re-throttle.
            wuburst = consts.tile([128, 512], f16, tag="wuburst")
            nc.gpsimd.memset(wuburst[:], 0.0)
            with tc.tile_pool(name="psW", bufs=1, space="PSUM") as psW:
                wups = psW.tile([128, 512], f32, tag="wups")
                for _ in range(30):
                    nc.tensor.matmul(wups[:], lhsT=u3s[0][0][:], rhs=wuburst[:],
                                     start=True, stop=True)

            # wflatT[kk, node*C + c] = sum_e attr[node, e] * Wcat[e, kk*C + c]
            nbC = nb * C
            wflatT = dpool.tile([KK, nbC], f16, tag="wflatT")

            # ---------------- phase A: produce wflatT ----------------
            with tc.tile_pool(name="pa", bufs=4) as pa, \
                 tc.tile_pool(name="psA", bufs=4, space="PSUM") as psA:
                attrT = pa.tile([E, nb], f16, tag="attrT")
                nc.sync.dma_start(out=attrT[:], in_=attrT_d[:])
                wcat = pa.tile([E, WROW], f16, tag="wcat")
                nc.sync.dma_start(out=wcat[:], in_=wcat_d[:])
                wflatT_ap = wflatT[:]
                for gs in range(0, nb, 128):
                    gn = min(128, nb - gs)
                    for j in range(WROW // 512):
                        pw = psA.tile([128, 512], f32, tag="pw")
                        nc.tensor.matmul(
                            pw[:gn],
                            lhsT=attrT[:, gs:gs + gn],
                            rhs=wcat[:, 512 * j:512 * (j + 1)],
                            start=True, stop=True,
                        )
                        wf = pa.tile([128, 512], f16, tag="wf")
                        nc.vector.tensor_copy(wf[:gn], pw[:gn])
                        # scatter-transpose: (node, 4 kk, c) -> wflatT rows
                        # SWDGE (gpsimd): HWDGE queue descriptors allow only
                        # one sync wait and this DMA needs two.
                        nc.gpsimd.dma_start(
                            out=bass.AP(
                                tensor=wflatT_ap.tensor,
                                offset=wflatT_ap.offset + 4 * j * nbC + gs * C,
                                ap=[[C, gn], [nbC, 4], [1, C]],
                            ),
                            in_=wf[:gn],
                        )

            # ---------------- phase B: main loop ----------------
            wflatT_ap = wflatT[:]
            embT_ap = embT_d[:]

            def wflat_gather(kk0, col0, kcnt, irep):
                """AP over wflatT: rows (k, i-rep), cols = F contiguous."""
                ap = [[nbC, kcnt]]
                if irep > 1:
                    ap.append([0, irep])
                ap += [[1, F]]
                return bass.AP(
                    tensor=wflatT_ap.tensor,
                    offset=wflatT_ap.offset + kk0 * nbC + col0,
                    ap=ap,
                )

            # Per-tile software pipeline, one stage per iteration lag so
            # every instruction's producers finished >=1 iteration earlier:
            #   load(t) -> G(t+1) -> mains(t+2) -> S(t+3) -> ysel(t+4)
            #   -> s2(t+5) -> xred(t+6) -> out(t+7)
            # A dependency-free matmul burst right after the barrier (and
            # periodically) pushes the PE HAM window to K=8/8; the loop has
            # no >=3.4us PE-idle window, so the clock stays warm.
            with tc.tile_pool(name="st", bufs=8) as st, \
                 tc.tile_pool(name="pP", bufs=4, space="PSUM") as pP, \
                 tc.tile_pool(name="pP1", bufs=2, space="PSUM") as pP1:
                state = {}

                def warm_burst(n):
                    wub = pP.tile([128, F], f32, tag="P", name="wub")
                    for _ in range(n):
                        nc.tensor.matmul(wub[:], lhsT=u3s[0][0][:],
                                         rhs=wuburst[:], start=True, stop=True)

                def stage_load(t):
                    node0 = TB * t
                    col0 = node0 * C
                    embT = st.tile([Y, F], f16, tag="embT")
                    nc.sync.dma_start(out=embT[:], in_=embT_d[:, col0:col0 + F])
                    embB = st.tile([128, F], f16, tag="embB")
                    nc.sync.dma_start(
                        out=embB[:],
                        in_=bass.AP(
                            tensor=embT_ap.tensor,
                            offset=embT_ap.offset + col0,
                            ap=[[0, 8], [nbC, Y], [1, F]],
                        ),
                    )
                    wm0 = st.tile([128, F], f16, tag="wm0")
                    nc.sync.dma_start(out=wm0[:], in_=wflat_gather(0, col0, 8, Y))
                    wm1 = st.tile([128, F], f16, tag="wm1")
                    nc.sync.dma_start(out=wm1[:], in_=wflat_gather(8, col0, 8, Y))
                    wm2 = st.tile([112, F], f16, tag="wm2")
                    nc.sync.dma_start(out=wm2[:], in_=wflat_gather(16, col0, 7, Y))
                    w1b = st.tile([Y, F], f16, tag="w1b")
                    nc.sync.dma_start(out=w1b[:], in_=wflat_gather(27, col0, 1, Y))
                    g2 = st.tile([116, F], f16, tag="g2")
                    nc.sync.dma_start(out=g2[112:116],
                                      in_=wflat_gather(23, col0, 4, 1))
                    state[t] = {"embT": embT, "embB": embB, "w1b": w1b,
                                "wm0": wm0, "wm1": wm1, "wm2": wm2, "g2": g2,
                                "node0": node0}

                def stage_g(t):
                    sd = state[t]
                    g0 = st.tile([128, F], f16, tag="g0")
                    nc.gpsimd.tensor_mul(g0[:], sd["embB"][:], sd["wm0"][:])
                    g1 = st.tile([128, F], f16, tag="g1")
                    nc.gpsimd.tensor_mul(g1[:], sd["embB"][:], sd["wm1"][:])
                    g2 = sd["g2"]
                    nc.gpsimd.tensor_mul(g2[:112], sd["embB"][:112], sd["wm2"][:])
                    sd["g"] = (g0, g1, g2)

                def stage_mains(t):
                    sd = state[t]
                    P = []
                    for h in range(2):
                        ph = pP.tile([128, F], f32, tag="P", name="Pt")
                        for m in range(3):
                            nc.tensor.matmul(
                                ph[:],
                                lhsT=u3s[h][m][:KM[m]],
                                rhs=sd["g"][m][:KM[m]],
                                start=(m == 0), stop=(m == 2),
                            )
                        P.append(ph)
                    sd["P"] = P

                def stage_s(t):
                    sd = state[t]
                    S = []
                    for h in range(2):
                        sh = st.tile([128, F], f16, tag=f"s{h}")
                        nc.vector.tensor_mul(sh[:], sd["P"][h][:], sd["embB"][:])
                        S.append(sh)
                    sd["S"] = S

                def stage_ysel(t):
                    sd = state[t]
                    p1 = pP1.tile([16, F], f32, tag="P1")
                    nc.tensor.matmul(p1[:], lhsT=sel[0][:], rhs=sd["S"][0][:],
                                     start=True, stop=False)
                    nc.tensor.matmul(p1[:], lhsT=sel[1][:], rhs=sd["S"][1][:],
                                     start=False, stop=True)
                    sd["p1"] = p1

                def stage_x(t):
                    sd = state[t]
                    s2 = st.tile([48, F], f16, tag="s2")
                    if t < 8:
                        # zero rows 16:32 once per pool slot (8 slots); the
                        # K=48 reduction multiplies them by zero weights
                        nc.gpsimd.memset(s2[:], 0.0)
                    nc.vector.tensor_mul(s2[:16], sd["p1"][:], sd["embT"][:])
                    nc.vector.tensor_mul(s2[32:48], sd["embT"][:],
                                         sd["w1b"][:])
                    sd["s2"] = s2

                def stage_xred(t):
                    # single K=32 reduction: rows 0:16 weighted by ones
                    # (sum_x o2*emb_x), rows 16:32 by U1 (U1-term)
                    sd = state[t]
                    p2 = pP1.tile([1, F], f32, tag="P2")
                    nc.tensor.matmul(p2[:], lhsT=onesu1[:], rhs=sd["s2"][:],
                                     start=True, stop=True)
                    sd["p2"] = p2

                def stage_out(t):
                    sd = state.pop(t)
                    o1 = st.tile([1, F], f32, tag="o1")
                    nc.scalar.copy(o1[:], sd["p2"][:])
                    nc.sync.dma_start(out=out_d[sd["node0"]:sd["node0"] + TB, :],
                                      in_=o1[:])

                def guard(f, t):
                    if 0 <= t < nt:
                        f(t)

                warm_burst(12)
                for u in range(nt + 7):
                    guard(stage_ysel, u - 4)
                    guard(stage_xred, u - 6)
                    guard(stage_load, u)
                    guard(stage_g, u - 1)
                    guard(stage_mains, u - 2)
                    guard(stage_s, u - 3)
                    guard(stage_x, u - 5)
                    guard(stage_out, u - 7)
    nc.compile()
    return nc


# ---------------- host-side input preparation ----------------

def _prep_constants(U3, U2, U1):
    """Stationary operands: U3/U2 reordered to (k-major ik rows, (x,y) cols)."""
    U3 = np.asarray(U3, dtype=np.float32)
    U2 = np.asarray(U2, dtype=np.float32)
    U1 = np.asarray(U1, dtype=np.float32)
    # rows r=(k,i)=k*16+i, cols (x,y)=x*16+y
    U3r = U3[0].transpose(3, 2, 0, 1).reshape(Z3 * Y, Y * Y)
    U2r = U2[0].transpose(2, 0, 1).reshape(Z2, Y * Y)
    M = np.vstack([U3r, U2r])                       # (372, 256)
    u3s = np.zeros((2, 3, 128, 128), dtype=np.float16)
    for m in range(3):
        chunk = M[128 * m:128 * m + KM[m]]
        for h in range(2):
            u3s[h, m, :KM[m], :] = chunk[:, 128 * h:128 * (h + 1)]
    sel = np.zeros((2, 128, 16), dtype=np.float16)
    for h in range(2):
        for p in range(128):
            sel[h, p, 8 * h + p // 16] = 1.0
    onesu1 = np.zeros((3 * Y, 1), dtype=np.float16)
    onesu1[:Y, 0] = 1.0
    onesu1[2 * Y:, 0] = U1[0, :, 0]
    return u3s, sel, onesu1


def _prep_core_inputs(emb_pad, attr_pad, wcat, consts, g, nb=NB):
    u3s, sel, onesu1 = consts
    sl = slice(g * nb, (g + 1) * nb)
    embT = np.ascontiguousarray(
        emb_pad[sl].transpose(2, 0, 1).reshape(Y, nb * C)
    ).astype(np.float16)
    attrT = np.ascontiguousarray(attr_pad[sl].T).astype(np.float16)
    return {
        "embT": embT,
        "attrT": attrT,
        "wcat": wcat,
        "u3s": u3s,
        "sel": sel,
        "onesu1": onesu1,
    }


def _prep_all(node_embeddings, node_attributes, U3, U2, U1, W3, W2, W1):
    emb = np.asarray(node_embeddings, dtype=np.float32)
    attr = np.asarray(node_attributes, dtype=np.float32)
    emb_pad = np.zeros((NPAD, C, Y), dtype=np.float32)
    emb_pad[:N] = emb
    attr_pad = np.zeros((NPAD, E), dtype=np.float32)
    attr_pad[:N] = attr
    # wcat[e, kk*C + c]: kk 0..22 = W3, 23..26 = W2, 27 = W1
    wcat = np.concatenate(
        [np.asarray(W3, np.float32), np.asarray(W2, np.float32),
         np.asarray(W1, np.float32)], axis=1
    ).reshape(E, WROW).astype(np.float16)
    consts = _prep_constants(U3, U2, U1)
    return [
        _prep_core_inputs(emb_pad, attr_pad, wcat, consts, g)
        for g in range(NCORES)
    ]


def kernel(node_embeddings, node_attributes, U3, U2, U1, W3, W2, W1):
    from concourse.bass_utils import run_bass_kernel_spmd

    if "nc" not in _CACHE:
        _CACHE["nc"] = _build_program(NB)
    nc = _CACHE["nc"]
    in_maps = _prep_all(node_embeddings, node_attributes,
                        U3, U2, U1, W3, W2, W1)
    trace = bool(int(os.environ.get("KERNEL_TRACE", "0")))
    res = run_bass_kernel_spmd(
        nc, in_maps, core_ids=list(range(NCORES)), trace=trace,
    )
    _CACHE["last_results"] = res
    out = np.concatenate([res.results[g]["out"] for g in range(NCORES)], axis=0)
    return np.ascontiguousarray(out[:N]).astype(np.float32)



# revision 21
# speedup vs baseline: 1.0841x; 1.0841x over previous
"""Trainium2 Bass kernel for the MACE-style symmetric contraction.

Math (per node b, feature c, with emb = node_embeddings[b, c, :] (16,)):
    w{3,2,1}[k, c] = sum_e attr[b, e] * W{3,2,1}[e, k, c]
    out3[x, y] = sum_{i,k} emb[i] * w3[k] * U3[0, x, y, i, k]        (16, 16)
    M3[x, y]   = out3[x, y] + sum_k2 U2[0, x, y, k2] * w2[k2]
    o2[x]      = sum_y M3[x, y] * emb[y] + U1[0, x, 0] * w1[0]
    o1         = sum_x o2[x] * emb[x]
    output[b, c] = o1

Mapping: columns = (node-in-tile, c) pairs, 4 nodes x 128 c = 512 cols/tile.
The (i, k) contraction (k-major, 368 rows + 4 U2 rows) runs on the PE as
3 accumulating matmuls per output half (x,y) -> 256 rows in 2 halves of 128.

v2 layout: all per-tile weight replication arrives in ONE DMA (wmall3,
[128, 1536]), G-build runs on gpsimd as one op, ACT evacuates PSUM, DVE
does SBUF muls; p1/p2 PSUM banks are shared 4 tiles wide via column-strip
tile positions; outputs drain once per quad.
"""

import os

import numpy as np

# ---------------- problem constants (hardcoded per contract) ----------------
N, C, Y, E = 3000, 128, 16, 10
Z3, Z2, Z1 = 23, 4, 1
NCORES = 8
NB = 376                # nodes per core (3008 = 8*376, padded)
NPAD = NCORES * NB
TB = 4                  # nodes per tile
F = TB * C              # 512 columns per tile
NT = NB // TB           # 94 tiles
KK = Z3 + Z2 + Z1       # 28 packed k rows in wflat (kk' = [W3 0..22, W1 23, W2 24..27])
WROW = KK * C           # 3584: wflat row length
KM = (128, 128, 116)    # contraction chunk K sizes (368 ik rows + 4 U2 rows)

_CACHE = {}


def _build_program(nb):
    """Build the single-core Bass program (SPMD: same program, all cores)."""
    import concourse.bass as bass
    import concourse.mybir as mybir
    import concourse.tile as tile
    from concourse import bacc

    f16, f32 = mybir.dt.float16, mybir.dt.float32
    nt = nb // TB
    nc = bacc.Bacc(None, target_bir_lowering=False)

    embT_d = nc.dram_tensor("embT", [Y, nb * C], f16, kind="ExternalInput")
    attrT_d = nc.dram_tensor("attrT", [E, nb], f16, kind="ExternalInput")
    wcat_d = nc.dram_tensor("wcat", [E, WROW], f16, kind="ExternalInput")
    u3s_d = nc.dram_tensor("u3s", [2, 3, 128, 128], f16, kind="ExternalInput")
    sel_d = nc.dram_tensor("sel", [2, 128, 16], f16, kind="ExternalInput")
    onesu1_d = nc.dram_tensor("onesu1", [48, 1], f16, kind="ExternalInput")
    out_d = nc.dram_tensor("out", [nb, C], f32, kind="ExternalOutput")

    nbC = nb * C

    with tile.TileContext(nc) as tc:
        with tc.tile_pool(name="consts", bufs=1) as consts, \
             tc.tile_pool(name="dram", bufs=1, space="DRAM") as dpool:
            # stationaries, loaded once
            u3s = []
            for h in range(2):
                row = []
                for m in range(3):
                    t = consts.tile([128, 128], f16, tag=f"u3s{h}{m}")
                    nc.sync.dma_start(out=t[:], in_=u3s_d[h, m])
                    row.append(t)
                u3s.append(row)
            sel = []
            for h in range(2):
                t = consts.tile([128, 16], f16, tag=f"sel{h}")
                nc.sync.dma_start(out=t[:], in_=sel_d[h])
                sel.append(t)
            onesu1 = consts.tile([48, 1], f16, tag="onesu1")
            nc.sync.dma_start(out=onesu1[:], in_=onesu1_d[:])

            # PE warm-up: dependency-free matmuls push the HAM activity
            # window to K=8/8 (2.4 GHz) before real work starts.
            wuburst = consts.tile([128, 512], f16, tag="wuburst")
            nc.gpsimd.memset(wuburst[:], 0.0)
            with tc.tile_pool(name="psW", bufs=1, space="PSUM") as psW:
                wups = psW.tile([128, 512], f32, tag="wups")
                for _ in range(30):
                    nc.tensor.matmul(wups[:], lhsT=u3s[0][0][:], rhs=wuburst[:],
                                     start=True, stop=True)

            # wflatT[kk, node*C + c] = sum_e attr[node, e] * Wcat[e, kk*C + c]
            wflatT = dpool.tile([KK, nbC], f16, tag="wflatT")

            # ---------------- phase A: produce wflatT ----------------
            with tc.tile_pool(name="pa", bufs=4) as pa, \
                 tc.tile_pool(name="psA", bufs=4, space="PSUM") as psA:
                attrT = pa.tile([E, nb], f16, tag="attrT")
                nc.sync.dma_start(out=attrT[:], in_=attrT_d[:])
                wcat = pa.tile([E, WROW], f16, tag="wcat")
                nc.sync.dma_start(out=wcat[:], in_=wcat_d[:])
                wflatT_ap = wflatT[:]
                for gs in range(0, nb, 128):
                    gn = min(128, nb - gs)
                    for j in range(WROW // 512):
                        pw = psA.tile([128, 512], f32, tag="pw")
                        nc.tensor.matmul(
                            pw[:gn],
                            lhsT=attrT[:, gs:gs + gn],
                            rhs=wcat[:, 512 * j:512 * (j + 1)],
                            start=True, stop=True,
                        )
                        wf = pa.tile([128, 512], f16, tag="wf")
                        nc.vector.tensor_copy(wf[:gn], pw[:gn])
                        # scatter-transpose: (node, 4 kk, c) -> wflatT rows
                        nc.gpsimd.dma_start(
                            out=bass.AP(
                                tensor=wflatT_ap.tensor,
                                offset=wflatT_ap.offset + 4 * j * nbC + gs * C,
                                ap=[[C, gn], [nbC, 4], [1, C]],
                            ),
                            in_=wf[:gn],
                        )

            # ---------------- phase B: main loop ----------------
            wflatT_ap = wflatT[:]
            embT_ap = embT_d[:]
            F4 = 4 * F
            nq = (nt + 3) // 4

            # Quad-granular loads and G ops (big ops amortize fixed costs),
            # per-tile matmul pipeline, quad-granular output drain.
            with tc.tile_pool(name="stq", bufs=4) as stq, \
                 tc.tile_pool(name="st", bufs=8) as st, \
                 tc.tile_pool(name="sto", bufs=2) as sto, \
                 tc.tile_pool(name="pP", bufs=2, space="PSUM") as pP, \
                 tc.tile_pool(name="pP1", bufs=2, space="PSUM") as pP1, \
                 tc.tile_pool(name="pP2", bufs=2, space="PSUM") as pP2:
                qstate = {}
                state = {}

                def stage_load(q):
                    col0 = 4 * q * F
                    qw = min(F4, nbC - col0)
                    # embB[p, f] = emb[p % 16, col0 + f]  (8x partition rep)
                    embB = stq.tile([128, F4], f16, tag="embB")
                    nc.sync.dma_start(
                        out=embB[:, 0:qw],
                        in_=bass.AP(
                            tensor=embT_ap.tensor,
                            offset=embT_ap.offset + col0,
                            ap=[[0, 8], [nbC, Y], [1, qw]],
                        ),
                    )
                    # wm{m}[p = kp*16+i, f] = wflat[8m + kp, col0 + f]
                    def wmdma(eng, kk0, rows):
                        wm = stq.tile([rows, F4], f16, tag=f"wm{kk0}",
                                      name=f"wm{kk0}")
                        eng.dma_start(
                            out=wm[:, 0:qw],
                            in_=bass.AP(
                                tensor=wflatT_ap.tensor,
                                offset=wflatT_ap.offset + kk0 * nbC + col0,
                                ap=[[nbC, rows // 16], [0, 16], [1, qw]],
                            ),
                        )
                        return wm
                    wm0 = wmdma(nc.sync, 0, 128)
                    wm1 = wmdma(nc.sync, 8, 128)
                    wm2 = wmdma(nc.scalar, 16, 128)
                    w1b = wmdma(nc.scalar, 23, 16)
                    # U2 rows (kk' 24..27) direct into g2q[112:116]
                    g2q = stq.tile([116, F4], f16, tag="g2q")
                    nc.scalar.dma_start(
                        out=g2q[112:116, 0:qw],
                        in_=bass.AP(
                            tensor=wflatT_ap.tensor,
                            offset=wflatT_ap.offset + 24 * nbC + col0,
                            ap=[[nbC, 4], [1, qw]],
                        ),
                    )
                    qstate[q] = {"embB": embB, "wm0": wm0, "wm1": wm1,
                                 "wm2": wm2, "w1b": w1b, "g2q": g2q,
                                 "qw": qw}

                def stage_ga(q):
                    qd = qstate[q]
                    qw = qd["qw"]
                    gA = stq.tile([128, F4], f16, tag="gA")
                    nc.gpsimd.tensor_mul(gA[:, 0:qw], qd["embB"][:, 0:qw],
                                         qd["wm0"][:, 0:qw])
                    gB = stq.tile([128, F4], f16, tag="gB")
                    nc.vector.tensor_mul(gB[:, 0:qw], qd["embB"][:, 0:qw],
                                         qd["wm1"][:, 0:qw])
                    qd["gA"], qd["gB"] = gA, gB

                def stage_gb(q):
                    qd = qstate[q]
                    qw = qd["qw"]
                    nc.vector.tensor_mul(qd["g2q"][0:112, 0:qw],
                                         qd["embB"][0:112, 0:qw],
                                         qd["wm2"][0:112, 0:qw])
                    s2q = stq.tile([48, F4], f16, tag="s2q")
                    if q < 5:
                        nc.gpsimd.memset(s2q[:], 0.0)
                    # rows 32:48: emb * w1 for the whole quad
                    nc.vector.tensor_mul(s2q[32:48, 0:qw],
                                         qd["embB"][0:16, 0:qw],
                                         qd["w1b"][:, 0:qw])
                    qd["s2q"] = s2q

                def stage_mains(t):
                    qd = qstate[t // 4]
                    j = t % 4
                    P = pP.tile([128, 2 * F], f32, tag="P")
                    rhs = (qd["gA"], qd["gB"], qd["g2q"])
                    for h in range(2):
                        for m in range(3):
                            nc.tensor.matmul(
                                P[:, F * h:F * (h + 1)],
                                lhsT=u3s[h][m][:KM[m]],
                                rhs=rhs[m][0:KM[m], F * j:F * (j + 1)],
                                start=(m == 0), stop=(m == 2),
                            )
                    state[t] = {"P": P}

                def stage_pc(t):
                    sd = state[t]
                    Pc = st.tile([128, 2 * F], f16, tag="Pc")
                    nc.scalar.copy(Pc[:], sd["P"][:])
                    sd["Pc"] = Pc

                def stage_s(t):
                    sd = state[t]
                    qd = qstate[t // 4]
                    j = t % 4
                    eB = qd["embB"][:, F * j:F * (j + 1)]
                    S = st.tile([128, 2 * F], f16, tag="S")
                    for h in range(2):
                        nc.vector.tensor_mul(S[:, F * h:F * (h + 1)],
                                             sd["Pc"][:, F * h:F * (h + 1)],
                                             eB)
                    sd["S"] = S

                def stage_ysel(t):
                    sd = state[t]
                    p1 = pP1.tile([16, F], f32, tag="p1", name="p1")
                    for h in range(2):
                        nc.tensor.matmul(p1[:], lhsT=sel[h][:],
                                         rhs=sd["S"][:, F * h:F * (h + 1)],
                                         start=(h == 0), stop=(h == 1))
                    sd["p1"] = p1

                def stage_s2(t):
                    sd = state[t]
                    qd = qstate[t // 4]
                    j = t % 4
                    nc.vector.tensor_mul(qd["s2q"][0:16, F * j:F * (j + 1)],
                                         sd["p1"][:],
                                         qd["embB"][0:16, F * j:F * (j + 1)])

                def stage_xred(t):
                    sd = state[t]
                    qd = qstate[t // 4]
                    j = t % 4
                    p2 = pP2.tile([1, F], f32, tag="p2", name="p2")
                    nc.tensor.matmul(p2[:], lhsT=onesu1[:],
                                     rhs=qd["s2q"][:, F * j:F * (j + 1)],
                                     start=True, stop=True)
                    sd["p2"] = p2

                def stage_o1(t):
                    sd = state[t]
                    qd = qstate[t // 4]
                    j = t % 4
                    if j == 0:
                        o1q = sto.tile([97, F], f32, tag="o1q")
                        qd["o1q"] = o1q
                    nc.scalar.copy(qd["o1q"][32 * j:32 * j + 1], sd["p2"][0:1])

                def stage_qout(t):
                    j = t % 4
                    if j != min(3, nt - 1 - (t - j)):
                        return
                    q = t // 4
                    qd = qstate[q]
                    nql = j + 1          # tiles in this quad
                    node0q = TB * (t - j)
                    oap = qd["o1q"][:]
                    nc.sync.dma_start(
                        out=out_d[node0q:node0q + TB * nql, :],
                        in_=bass.AP(
                            tensor=oap.tensor,
                            offset=oap.offset,
                            ap=[[32 * F, nql], [1, F]],
                        ),
                    )
                    for tt in range(t - j, t + 1):
                        state.pop(tt, None)

                def guard(f, t):
                    if 0 <= t < nt:
                        f(t)

                def qguard(f, q):
                    if 0 <= q < nq:
                        f(q)

                for u in range(nt + 14):
                    guard(stage_ysel, u - 9)
                    guard(stage_xred, u - 11)
                    if u % 4 == 0:
                        qguard(stage_load, u // 4)
                    elif u % 4 == 2:
                        qguard(stage_ga, u // 4)
                    elif u % 4 == 3:
                        qguard(stage_gb, u // 4)
                    guard(stage_mains, u - 6)
                    guard(stage_pc, u - 7)
                    guard(stage_s, u - 8)
                    guard(stage_s2, u - 10)
                    guard(stage_o1, u - 12)
                    guard(stage_qout, u - 13)
    nc.compile()
    return nc


# ---------------- host-side input preparation ----------------

def _prep_constants(U3, U2, U1):
    """Stationary operands: U3/U2 reordered to (k-major ik rows, (x,y) cols)."""
    U3 = np.asarray(U3, dtype=np.float32)
    U2 = np.asarray(U2, dtype=np.float32)
    U1 = np.asarray(U1, dtype=np.float32)
    # rows r=(k,i)=k*16+i, cols (x,y)=x*16+y
    U3r = U3[0].transpose(3, 2, 0, 1).reshape(Z3 * Y, Y * Y)
    U2r = U2[0].transpose(2, 0, 1).reshape(Z2, Y * Y)
    M = np.vstack([U3r, U2r])                       # (372, 256)
    u3s = np.zeros((2, 3, 128, 128), dtype=np.float16)
    for m in range(3):
        chunk = M[128 * m:128 * m + KM[m]]
        for h in range(2):
            u3s[h, m, :KM[m], :] = chunk[:, 128 * h:128 * (h + 1)]
    sel = np.zeros((2, 128, 16), dtype=np.float16)
    for h in range(2):
        for p in range(128):
            sel[h, p, 8 * h + p // 16] = 1.0
    onesu1 = np.zeros((3 * Y, 1), dtype=np.float16)
    onesu1[:Y, 0] = 1.0
    onesu1[2 * Y:, 0] = U1[0, :, 0]
    return u3s, sel, onesu1


def _prep_core_inputs(emb_pad, attr_pad, wcat, consts, g, nb=NB):
    u3s, sel, onesu1 = consts
    sl = slice(g * nb, (g + 1) * nb)
    embT = np.ascontiguousarray(
        emb_pad[sl].transpose(2, 0, 1).reshape(Y, nb * C)
    ).astype(np.float16)
    attrT = np.ascontiguousarray(attr_pad[sl].T).astype(np.float16)
    return {
        "embT": embT,
        "attrT": attrT,
        "wcat": wcat,
        "u3s": u3s,
        "sel": sel,
        "onesu1": onesu1,
    }


def _prep_all(node_embeddings, node_attributes, U3, U2, U1, W3, W2, W1):
    emb = np.asarray(node_embeddings, dtype=np.float32)
    attr = np.asarray(node_attributes, dtype=np.float32)
    emb_pad = np.zeros((NPAD, C, Y), dtype=np.float32)
    emb_pad[:N] = emb
    attr_pad = np.zeros((NPAD, E), dtype=np.float32)
    attr_pad[:N] = attr
    # wcat[e, kk*C + c]: kk' 0..22 = W3, 23 = W1, 24..27 = W2
    wcat = np.concatenate(
        [np.asarray(W3, np.float32), np.asarray(W1, np.float32),
         np.asarray(W2, np.float32)], axis=1
    ).reshape(E, WROW).astype(np.float16)
    consts = _prep_constants(U3, U2, U1)
    return [
        _prep_core_inputs(emb_pad, attr_pad, wcat, consts, g)
        for g in range(NCORES)
    ]


def kernel(node_embeddings, node_attributes, U3, U2, U1, W3, W2, W1):
    from concourse.bass_utils import run_bass_kernel_spmd

    if "nc" not in _CACHE:
        _CACHE["nc"] = _build_program(NB)
    nc = _CACHE["nc"]
    in_maps = _prep_all(node_embeddings, node_attributes,
                        U3, U2, U1, W3, W2, W1)
    trace = bool(int(os.environ.get("KERNEL_TRACE", "0")))
    res = run_bass_kernel_spmd(
        nc, in_maps, core_ids=list(range(NCORES)), trace=trace,
    )
    _CACHE["last_results"] = res
    out = np.concatenate([res.results[g]["out"] for g in range(NCORES)], axis=0)
    return np.ascontiguousarray(out[:N]).astype(np.float32)


# revision 23
# speedup vs baseline: 1.0952x; 1.0103x over previous
"""Trainium2 Bass kernel for the MACE-style symmetric contraction.

Math (per node b, feature c, with emb = node_embeddings[b, c, :] (16,)):
    w{3,2,1}[k, c] = sum_e attr[b, e] * W{3,2,1}[e, k, c]
    out3[x, y] = sum_{i,k} emb[i] * w3[k] * U3[0, x, y, i, k]        (16, 16)
    M3[x, y]   = out3[x, y] + sum_k2 U2[0, x, y, k2] * w2[k2]
    o2[x]      = sum_y M3[x, y] * emb[y] + U1[0, x, 0] * w1[0]
    o1         = sum_x o2[x] * emb[x]
    output[b, c] = o1

Mapping: columns = (node-in-tile, c) pairs, 4 nodes x 128 c = 512 cols/tile.
The (i, k) contraction (k-major, 368 rows + 4 U2 rows) runs on the PE as
3 accumulating matmuls per output half (x,y) -> 256 rows in 2 halves of 128.

v2 layout: all per-tile weight replication arrives in ONE DMA (wmall3,
[128, 1536]), G-build runs on gpsimd as one op, ACT evacuates PSUM, DVE
does SBUF muls; p1/p2 PSUM banks are shared 4 tiles wide via column-strip
tile positions; outputs drain once per quad.
"""

import os

import numpy as np

# ---------------- problem constants (hardcoded per contract) ----------------
N, C, Y, E = 3000, 128, 16, 10
Z3, Z2, Z1 = 23, 4, 1
NCORES = 8
NB = 376                # nodes per core (3008 = 8*376, padded)
NPAD = NCORES * NB
TB = 4                  # nodes per tile
F = TB * C              # 512 columns per tile
NT = NB // TB           # 94 tiles
KK = Z3 + Z2 + Z1       # 28 packed k rows in wflat (kk' = [W3 0..22, W1 23, W2 24..27])
WROW = KK * C           # 3584: wflat row length
KM = (128, 128, 116)    # contraction chunk K sizes (368 ik rows + 4 U2 rows)

_CACHE = {}


def _build_program(nb):
    """Build the single-core Bass program (SPMD: same program, all cores)."""
    import concourse.bass as bass
    import concourse.mybir as mybir
    import concourse.tile as tile
    from concourse import bacc

    f16, f32 = mybir.dt.float16, mybir.dt.float32
    nt = nb // TB
    nc = bacc.Bacc(None, target_bir_lowering=False)

    embT_d = nc.dram_tensor("embT", [Y, nb * C], f16, kind="ExternalInput")
    attrT_d = nc.dram_tensor("attrT", [E, nb], f16, kind="ExternalInput")
    wcat_d = nc.dram_tensor("wcat", [E, WROW], f16, kind="ExternalInput")
    u3s_d = nc.dram_tensor("u3s", [2, 3, 128, 128], f16, kind="ExternalInput")
    sel_d = nc.dram_tensor("sel", [2, 128, 16], f16, kind="ExternalInput")
    onesu1_d = nc.dram_tensor("onesu1", [112, 2], f16, kind="ExternalInput")
    out_d = nc.dram_tensor("out", [nb, C], f32, kind="ExternalOutput")

    nbC = nb * C

    with tile.TileContext(nc) as tc:
        with tc.tile_pool(name="consts", bufs=1) as consts, \
             tc.tile_pool(name="dram", bufs=1, space="DRAM") as dpool:
            # stationaries, loaded once
            u3s = []
            for h in range(2):
                row = []
                for m in range(3):
                    t = consts.tile([128, 128], f16, tag=f"u3s{h}{m}")
                    nc.sync.dma_start(out=t[:], in_=u3s_d[h, m])
                    row.append(t)
                u3s.append(row)
            sel = []
            for h in range(2):
                t = consts.tile([128, 16], f16, tag=f"sel{h}")
                nc.sync.dma_start(out=t[:], in_=sel_d[h])
                sel.append(t)
            onesu1 = consts.tile([112, 2], f16, tag="onesu1")
            nc.sync.dma_start(out=onesu1[:], in_=onesu1_d[:])

            # PE warm-up: dependency-free matmuls push the HAM activity
            # window to K=8/8 (2.4 GHz) before real work starts.
            wuburst = consts.tile([128, 512], f16, tag="wuburst")
            nc.gpsimd.memset(wuburst[:], 0.0)
            with tc.tile_pool(name="psW", bufs=1, space="PSUM") as psW:
                wups = psW.tile([128, 512], f32, tag="wups")
                for _ in range(30):
                    nc.tensor.matmul(wups[:], lhsT=u3s[0][0][:], rhs=wuburst[:],
                                     start=True, stop=True)

            # wflatT[kk, node*C + c] = sum_e attr[node, e] * Wcat[e, kk*C + c]
            wflatT = dpool.tile([KK, nbC], f16, tag="wflatT")

            # ---------------- phase A: produce wflatT ----------------
            with tc.tile_pool(name="pa", bufs=4) as pa, \
                 tc.tile_pool(name="psA", bufs=4, space="PSUM") as psA:
                attrT = pa.tile([E, nb], f16, tag="attrT")
                nc.sync.dma_start(out=attrT[:], in_=attrT_d[:])
                wcat = pa.tile([E, WROW], f16, tag="wcat")
                nc.sync.dma_start(out=wcat[:], in_=wcat_d[:])
                wflatT_ap = wflatT[:]
                for gs in range(0, nb, 128):
                    gn = min(128, nb - gs)
                    for j in range(WROW // 512):
                        pw = psA.tile([128, 512], f32, tag="pw")
                        nc.tensor.matmul(
                            pw[:gn],
                            lhsT=attrT[:, gs:gs + gn],
                            rhs=wcat[:, 512 * j:512 * (j + 1)],
                            start=True, stop=True,
                        )
                        wf = pa.tile([128, 512], f16, tag="wf")
                        nc.vector.tensor_copy(wf[:gn], pw[:gn])
                        # scatter-transpose: (node, 4 kk, c) -> wflatT rows
                        nc.gpsimd.dma_start(
                            out=bass.AP(
                                tensor=wflatT_ap.tensor,
                                offset=wflatT_ap.offset + 4 * j * nbC + gs * C,
                                ap=[[C, gn], [nbC, 4], [1, C]],
                            ),
                            in_=wf[:gn],
                        )

            # ---------------- phase B: main loop ----------------
            wflatT_ap = wflatT[:]
            embT_ap = embT_d[:]
            F4 = 4 * F
            nq = (nt + 3) // 4

            # Quad-granular loads and G ops (big ops amortize fixed costs),
            # per-tile matmul pipeline, quad-granular output drain.
            with tc.tile_pool(name="stq", bufs=4) as stq, \
                 tc.tile_pool(name="st", bufs=8) as st, \
                 tc.tile_pool(name="sto", bufs=2) as sto, \
                 tc.tile_pool(name="pP", bufs=2, space="PSUM") as pP, \
                 tc.tile_pool(name="pP1", bufs=2, space="PSUM") as pP1, \
                 tc.tile_pool(name="pP2", bufs=2, space="PSUM") as pP2:
                qstate = {}
                state = {}

                def stage_load(q):
                    col0 = 4 * q * F
                    qw = min(F4, nbC - col0)
                    # embB[p, f] = emb[p % 16, col0 + f]  (8x partition rep)
                    embB = stq.tile([128, F4], f16, tag="embB")
                    nc.sync.dma_start(
                        out=embB[:, 0:qw],
                        in_=bass.AP(
                            tensor=embT_ap.tensor,
                            offset=embT_ap.offset + col0,
                            ap=[[0, 8], [nbC, Y], [1, qw]],
                        ),
                    )
                    # wm{m}[p = kp*16+i, f] = wflat[8m + kp, col0 + f]
                    def wmdma(eng, kk0, rows):
                        wm = stq.tile([rows, F4], f16, tag=f"wm{kk0}",
                                      name=f"wm{kk0}")
                        eng.dma_start(
                            out=wm[:, 0:qw],
                            in_=bass.AP(
                                tensor=wflatT_ap.tensor,
                                offset=wflatT_ap.offset + kk0 * nbC + col0,
                                ap=[[nbC, rows // 16], [0, 16], [1, qw]],
                            ),
                        )
                        return wm
                    wm0 = wmdma(nc.sync, 0, 128)
                    wm1 = wmdma(nc.sync, 8, 128)
                    wm2 = wmdma(nc.scalar, 16, 128)
                    w1b = wmdma(nc.scalar, 23, 16)
                    # U2 rows (kk' 24..27) direct into g2q[112:116]
                    g2q = stq.tile([116, F4], f16, tag="g2q")
                    nc.scalar.dma_start(
                        out=g2q[112:116, 0:qw],
                        in_=bass.AP(
                            tensor=wflatT_ap.tensor,
                            offset=wflatT_ap.offset + 24 * nbC + col0,
                            ap=[[nbC, 4], [1, qw]],
                        ),
                    )
                    qstate[q] = {"embB": embB, "wm0": wm0, "wm1": wm1,
                                 "wm2": wm2, "w1b": w1b, "g2q": g2q,
                                 "qw": qw}

                def stage_ga(q):
                    qd = qstate[q]
                    qw = qd["qw"]
                    gA = stq.tile([128, F4], f16, tag="gA")
                    nc.gpsimd.tensor_mul(gA[:, 0:qw], qd["embB"][:, 0:qw],
                                         qd["wm0"][:, 0:qw])
                    gB = stq.tile([128, F4], f16, tag="gB")
                    nc.vector.tensor_mul(gB[:, 0:qw], qd["embB"][:, 0:qw],
                                         qd["wm1"][:, 0:qw])
                    qd["gA"], qd["gB"] = gA, gB

                def stage_gb(q):
                    qd = qstate[q]
                    qw = qd["qw"]
                    nc.vector.tensor_mul(qd["g2q"][0:112, 0:qw],
                                         qd["embB"][0:112, 0:qw],
                                         qd["wm2"][0:112, 0:qw])

                def stage_mains(t):
                    qd = qstate[t // 4]
                    j = t % 4
                    P = pP.tile([128, 2 * F], f32, tag="P")
                    rhs = (qd["gA"], qd["gB"], qd["g2q"])
                    for h in range(2):
                        for m in range(3):
                            nc.tensor.matmul(
                                P[:, F * h:F * (h + 1)],
                                lhsT=u3s[h][m][:KM[m]],
                                rhs=rhs[m][0:KM[m], F * j:F * (j + 1)],
                                start=(m == 0), stop=(m == 2),
                            )
                    state[t] = {"P": P}

                def stage_pc(t):
                    sd = state[t]
                    Pc = st.tile([128, 2 * F], f16, tag="Pc")
                    nc.scalar.copy(Pc[:], sd["P"][:])
                    sd["Pc"] = Pc

                def stage_s(t):
                    sd = state[t]
                    qd = qstate[t // 4]
                    j = t % 4
                    eB = qd["embB"][:, F * j:F * (j + 1)]
                    S = st.tile([128, 2 * F], f16, tag="S")
                    for h in range(2):
                        nc.vector.tensor_mul(S[:, F * h:F * (h + 1)],
                                             sd["Pc"][:, F * h:F * (h + 1)],
                                             eB)
                    sd["S"] = S

                def stage_ysel(t):
                    sd = state[t]
                    p1 = pP1.tile([16, F], f32, tag="p1", name="p1")
                    for h in range(2):
                        nc.tensor.matmul(p1[:], lhsT=sel[h][:],
                                         rhs=sd["S"][:, F * h:F * (h + 1)],
                                         start=(h == 0), stop=(h == 1))
                    sd["p1"] = p1

                def stage_s2(t):
                    sd = state[t]
                    qd = qstate[t // 4]
                    j = t % 4
                    j2 = j % 2
                    if j2 == 0:
                        s2p = st.tile([112, F], f16, tag="s2p")
                        if t < 16:
                            nc.gpsimd.memset(s2p[:], 0.0)
                        state[t]["s2p"] = s2p
                        # both tiles' w1 rows for this pair
                        for jj in (j, j + 1):
                            if t - j + jj < nt:
                                nc.vector.tensor_mul(
                                    s2p[64 * (jj % 2) + 32:64 * (jj % 2) + 48],
                                    qd["embB"][0:16, F * jj:F * (jj + 1)],
                                    qd["w1b"][:, F * jj:F * (jj + 1)])
                    else:
                        s2p = state[t - 1]["s2p"]
                        state[t]["s2p"] = s2p
                    nc.vector.tensor_mul(s2p[64 * j2:64 * j2 + 16],
                                         sd["p1"][:],
                                         qd["embB"][0:16, F * j:F * (j + 1)])

                def stage_xred(t):
                    # one M=2 matmul per pair, fired at the odd tile
                    if t % 2 == 0 and t + 1 < nt:
                        return
                    sd = state[t]
                    p2 = pP2.tile([2, F], f32, tag="p2", name="p2")
                    nc.tensor.matmul(p2[:], lhsT=onesu1[:],
                                     rhs=sd["s2p"][:],
                                     start=True, stop=True)
                    sd["p2"] = p2

                def stage_o1(t):
                    if t % 2 == 0 and t + 1 < nt:
                        return
                    sd = state[t]
                    nr = t % 2 + 1
                    o1p = sto.tile([2, F], f32, tag="o1p")
                    nc.scalar.copy(o1p[0:nr], sd["p2"][0:nr])
                    sd["o1p"] = o1p

                def stage_qout(t):
                    # per-pair output store, fired at the odd tile (or tail)
                    if t % 2 == 0 and t + 1 < nt:
                        return
                    j2 = t % 2
                    nr = j2 + 1
                    node0p = TB * (t - j2)
                    sd = state[t]
                    nc.sync.dma_start(
                        out=out_d[node0p:node0p + TB * nr, :],
                        in_=sd["o1p"][0:nr],
                    )
                    state.pop(t, None)
                    state.pop(t - 1, None)

                def guard(f, t):
                    if 0 <= t < nt:
                        f(t)

                def qguard(f, q):
                    if 0 <= q < nq:
                        f(q)

                for u in range(nt + 14):
                    guard(stage_ysel, u - 9)
                    guard(stage_xred, u - 11)
                    if u % 4 == 0:
                        qguard(stage_load, u // 4)
                    elif u % 4 == 2:
                        qguard(stage_ga, u // 4)
                    elif u % 4 == 3:
                        qguard(stage_gb, u // 4)
                    guard(stage_mains, u - 6)
                    guard(stage_pc, u - 7)
                    guard(stage_s, u - 8)
                    guard(stage_s2, u - 10)
                    guard(stage_o1, u - 12)
                    guard(stage_qout, u - 13)
    nc.compile()
    return nc


# ---------------- host-side input preparation ----------------

def _prep_constants(U3, U2, U1):
    """Stationary operands: U3/U2 reordered to (k-major ik rows, (x,y) cols)."""
    U3 = np.asarray(U3, dtype=np.float32)
    U2 = np.asarray(U2, dtype=np.float32)
    U1 = np.asarray(U1, dtype=np.float32)
    # rows r=(k,i)=k*16+i, cols (x,y)=x*16+y
    U3r = U3[0].transpose(3, 2, 0, 1).reshape(Z3 * Y, Y * Y)
    U2r = U2[0].transpose(2, 0, 1).reshape(Z2, Y * Y)
    M = np.vstack([U3r, U2r])                       # (372, 256)
    u3s = np.zeros((2, 3, 128, 128), dtype=np.float16)
    for m in range(3):
        chunk = M[128 * m:128 * m + KM[m]]
        for h in range(2):
            u3s[h, m, :KM[m], :] = chunk[:, 128 * h:128 * (h + 1)]
    sel = np.zeros((2, 128, 16), dtype=np.float16)
    for h in range(2):
        for p in range(128):
            sel[h, p, 8 * h + p // 16] = 1.0
    # pair-stacked xred weights: [112, 2], col j2 covers rows 64*j2 + [0:16]
    # (ones) and 64*j2 + [32:48] (U1)
    onesu1 = np.zeros((112, 2), dtype=np.float16)
    for j2 in range(2):
        onesu1[64 * j2:64 * j2 + Y, j2] = 1.0
        onesu1[64 * j2 + 2 * Y:64 * j2 + 3 * Y, j2] = U1[0, :, 0]
    return u3s, sel, onesu1


def _prep_core_inputs(emb_pad, attr_pad, wcat, consts, g, nb=NB):
    u3s, sel, onesu1 = consts
    sl = slice(g * nb, (g + 1) * nb)
    embT = np.ascontiguousarray(
        emb_pad[sl].transpose(2, 0, 1).reshape(Y, nb * C)
    ).astype(np.float16)
    attrT = np.ascontiguousarray(attr_pad[sl].T).astype(np.float16)
    return {
        "embT": embT,
        "attrT": attrT,
        "wcat": wcat,
        "u3s": u3s,
        "sel": sel,
        "onesu1": onesu1,
    }


def _prep_all(node_embeddings, node_attributes, U3, U2, U1, W3, W2, W1):
    emb = np.asarray(node_embeddings, dtype=np.float32)
    attr = np.asarray(node_attributes, dtype=np.float32)
    emb_pad = np.zeros((NPAD, C, Y), dtype=np.float32)
    emb_pad[:N] = emb
    attr_pad = np.zeros((NPAD, E), dtype=np.float32)
    attr_pad[:N] = attr
    # wcat[e, kk*C + c]: kk' 0..22 = W3, 23 = W1, 24..27 = W2
    wcat = np.concatenate(
        [np.asarray(W3, np.float32), np.asarray(W1, np.float32),
         np.asarray(W2, np.float32)], axis=1
    ).reshape(E, WROW).astype(np.float16)
    consts = _prep_constants(U3, U2, U1)
    return [
        _prep_core_inputs(emb_pad, attr_pad, wcat, consts, g)
        for g in range(NCORES)
    ]


def kernel(node_embeddings, node_attributes, U3, U2, U1, W3, W2, W1):
    from concourse.bass_utils import run_bass_kernel_spmd

    if "nc" not in _CACHE:
        _CACHE["nc"] = _build_program(NB)
    nc = _CACHE["nc"]
    in_maps = _prep_all(node_embeddings, node_attributes,
                        U3, U2, U1, W3, W2, W1)
    trace = bool(int(os.environ.get("KERNEL_TRACE", "0")))
    res = run_bass_kernel_spmd(
        nc, in_maps, core_ids=list(range(NCORES)), trace=trace,
    )
    _CACHE["last_results"] = res
    out = np.concatenate([res.results[g]["out"] for g in range(NCORES)], axis=0)
    return np.ascontiguousarray(out[:N]).astype(np.float32)


# revision 25
# speedup vs baseline: 1.4163x; 1.2932x over previous
"""Trainium2 Bass kernel for the MACE-style symmetric contraction.

Math (per node b, feature c, with emb = node_embeddings[b, c, :] (16,)):
    w{3,2,1}[k, c] = sum_e attr[b, e] * W{3,2,1}[e, k, c]
    out3[x, y] = sum_{i,k} emb[i] * w3[k] * U3[0, x, y, i, k]        (16, 16)
    M3[x, y]   = out3[x, y] + sum_k2 U2[0, x, y, k2] * w2[k2]
    o2[x]      = sum_y M3[x, y] * emb[y] + U1[0, x, 0] * w1[0]
    o1         = sum_x o2[x] * emb[x]
    output[b, c] = o1

Mapping: columns = (node-in-tile, c) pairs, 4 nodes x 128 c = 512 cols/tile.
The (i, k) contraction (k-major, 368 rows + 4 U2 rows) runs on the PE as
3 accumulating matmuls per output half (x,y) -> 256 rows in 2 halves of 128.

v2 layout: all per-tile weight replication arrives in ONE DMA (wmall3,
[128, 1536]), G-build runs on gpsimd as one op, ACT evacuates PSUM, DVE
does SBUF muls; p1/p2 PSUM banks are shared 4 tiles wide via column-strip
tile positions; outputs drain once per quad.
"""

import os

import numpy as np

# ---------------- problem constants (hardcoded per contract) ----------------
N, C, Y, E = 3000, 128, 16, 10
Z3, Z2, Z1 = 23, 4, 1
NCORES = 8
NB = 376                # nodes per core (3008 = 8*376, padded)
NPAD = NCORES * NB
TB = 4                  # nodes per tile
F = TB * C              # 512 columns per tile
NT = NB // TB           # 94 tiles
KK = Z3 + Z2 + Z1       # 28 packed k rows in wflat (kk' = [W3 0..22, W1 23, W2 24..27])
WROW = KK * C           # 3584: wflat row length
KM = (128, 128, 116)    # contraction chunk K sizes (368 ik rows + 4 U2 rows)

_CACHE = {}


def _build_program(nb):
    """Build the single-core Bass program (SPMD: same program, all cores)."""
    import concourse.bass as bass
    import concourse.mybir as mybir
    import concourse.tile as tile
    from concourse import bacc

    f16, f32 = mybir.dt.float16, mybir.dt.float32
    nt = nb // TB
    nc = bacc.Bacc(None, target_bir_lowering=False)

    embT_d = nc.dram_tensor("embT", [Y, nb * C], f16, kind="ExternalInput")
    attrT_d = nc.dram_tensor("attrT", [E, nb], f16, kind="ExternalInput")
    wcat_d = nc.dram_tensor("wcat", [E, WROW], f16, kind="ExternalInput")
    u3s_d = nc.dram_tensor("u3s", [2, 3, 128, 128], f16, kind="ExternalInput")
    sel_d = nc.dram_tensor("sel", [2, 128, 16], f16, kind="ExternalInput")
    onesu1_d = nc.dram_tensor("onesu1", [96, 2], f16, kind="ExternalInput")
    out_d = nc.dram_tensor("out", [nb, C], f32, kind="ExternalOutput")

    nbC = nb * C

    with tile.TileContext(nc) as tc:
        with tc.tile_pool(name="consts", bufs=1) as consts, \
             tc.tile_pool(name="dram", bufs=1, space="DRAM") as dpool:
            # stationaries, loaded once
            u3s = []
            for h in range(2):
                row = []
                for m in range(3):
                    t = consts.tile([128, 128], f16, tag=f"u3s{h}{m}")
                    nc.sync.dma_start(out=t[:], in_=u3s_d[h, m])
                    row.append(t)
                u3s.append(row)
            sel = []
            for h in range(2):
                t = consts.tile([128, 16], f16, tag=f"sel{h}")
                nc.sync.dma_start(out=t[:], in_=sel_d[h])
                sel.append(t)
            onesu1 = consts.tile([96, 2], f16, tag="onesu1")
            nc.sync.dma_start(out=onesu1[:], in_=onesu1_d[:])

            # PE warm-up: dependency-free matmuls push the HAM activity
            # window to K=8/8 (2.4 GHz) before real work starts.
            wuburst = consts.tile([128, 512], f16, tag="wuburst")
            nc.gpsimd.memset(wuburst[:], 0.0)
            with tc.tile_pool(name="psW", bufs=1, space="PSUM") as psW:
                wups = psW.tile([128, 512], f32, tag="wups")
                for _ in range(30):
                    nc.tensor.matmul(wups[:], lhsT=u3s[0][0][:], rhs=wuburst[:],
                                     start=True, stop=True)

            # wflatT[kk, node*C + c] = sum_e attr[node, e] * Wcat[e, kk*C + c]
            wflatT = dpool.tile([KK, nbC], f16, tag="wflatT")

            # ---------------- phase A: produce wflatT ----------------
            with tc.tile_pool(name="pa", bufs=4) as pa, \
                 tc.tile_pool(name="psA", bufs=4, space="PSUM") as psA:
                attrT = pa.tile([E, nb], f16, tag="attrT")
                nc.sync.dma_start(out=attrT[:], in_=attrT_d[:])
                wcat = pa.tile([E, WROW], f16, tag="wcat")
                nc.sync.dma_start(out=wcat[:], in_=wcat_d[:])
                wflatT_ap = wflatT[:]
                for gs in range(0, nb, 128):
                    gn = min(128, nb - gs)
                    for j in range(WROW // 512):
                        pw = psA.tile([128, 512], f32, tag="pw")
                        nc.tensor.matmul(
                            pw[:gn],
                            lhsT=attrT[:, gs:gs + gn],
                            rhs=wcat[:, 512 * j:512 * (j + 1)],
                            start=True, stop=True,
                        )
                        wf = pa.tile([128, 512], f16, tag="wf")
                        nc.vector.tensor_copy(wf[:gn], pw[:gn])
                        # scatter-transpose: (node, 4 kk, c) -> wflatT rows
                        nc.gpsimd.dma_start(
                            out=bass.AP(
                                tensor=wflatT_ap.tensor,
                                offset=wflatT_ap.offset + 4 * j * nbC + gs * C,
                                ap=[[C, gn], [nbC, 4], [1, C]],
                            ),
                            in_=wf[:gn],
                        )

            # ---------------- phase B: main loop ----------------
            wflatT_ap = wflatT[:]
            embT_ap = embT_d[:]
            F4 = 4 * F
            nq = (nt + 3) // 4

            # Quad-granular loads and G ops (big ops amortize fixed costs),
            # per-tile matmul pipeline, quad-granular output drain.
            with tc.tile_pool(name="stq", bufs=4) as stq, \
                 tc.tile_pool(name="st", bufs=8) as st, \
                 tc.tile_pool(name="sto", bufs=2) as sto, \
                 tc.tile_pool(name="pP", bufs=2, space="PSUM") as pP, \
                 tc.tile_pool(name="pP1", bufs=2, space="PSUM") as pP1, \
                 tc.tile_pool(name="pP2", bufs=2, space="PSUM") as pP2:
                qstate = {}
                state = {}

                def stage_load(q):
                    col0 = 4 * q * F
                    qw = min(F4, nbC - col0)
                    # embB[p, f] = emb[p % 16, col0 + f]  (8x partition rep)
                    embB = stq.tile([128, F4], f16, tag="embB")
                    nc.sync.dma_start(
                        out=embB[:, 0:qw],
                        in_=bass.AP(
                            tensor=embT_ap.tensor,
                            offset=embT_ap.offset + col0,
                            ap=[[0, 8], [nbC, Y], [1, qw]],
                        ),
                    )
                    # wm{m}[p = kp*16+i, f] = wflat[8m + kp, col0 + f]
                    def wmdma(eng, kk0, rows):
                        wm = stq.tile([rows, F4], f16, tag=f"wm{kk0}",
                                      name=f"wm{kk0}")
                        eng.dma_start(
                            out=wm[:, 0:qw],
                            in_=bass.AP(
                                tensor=wflatT_ap.tensor,
                                offset=wflatT_ap.offset + kk0 * nbC + col0,
                                ap=[[nbC, rows // 16], [0, 16], [1, qw]],
                            ),
                        )
                        return wm
                    wm0 = wmdma(nc.sync, 0, 128)
                    wm1 = wmdma(nc.sync, 8, 128)
                    wm2 = wmdma(nc.sync, 16, 128)
                    qn = qw // F
                    # embB4[16*jj + i, f] = emb[i, col0 + F*jj + f]
                    embB4 = stq.tile([64, F], f16, tag="embB4")
                    nc.scalar.dma_start(
                        out=embB4[0:16 * qn],
                        in_=bass.AP(
                            tensor=embT_ap.tensor,
                            offset=embT_ap.offset + col0,
                            ap=[[F, qn], [nbC, 16], [1, F]],
                        ),
                    )
                    # w1b4[16*jj + i, f] = wflat[23, col0 + F*jj + f]
                    w1b4 = stq.tile([64, F], f16, tag="w1b4")
                    nc.scalar.dma_start(
                        out=w1b4[0:16 * qn],
                        in_=bass.AP(
                            tensor=wflatT_ap.tensor,
                            offset=wflatT_ap.offset + 23 * nbC + col0,
                            ap=[[F, qn], [0, 16], [1, F]],
                        ),
                    )
                    # U2 rows (kk' 24..27) direct into g2q[112:116]
                    g2q = stq.tile([116, F4], f16, tag="g2q")
                    nc.scalar.dma_start(
                        out=g2q[112:116, 0:qw],
                        in_=bass.AP(
                            tensor=wflatT_ap.tensor,
                            offset=wflatT_ap.offset + 24 * nbC + col0,
                            ap=[[nbC, 4], [1, qw]],
                        ),
                    )
                    qstate[q] = {"embB": embB, "wm0": wm0, "wm1": wm1,
                                 "wm2": wm2, "embB4": embB4, "w1b4": w1b4,
                                 "g2q": g2q, "qw": qw}

                def stage_ga(q):
                    qd = qstate[q]
                    qw = qd["qw"]
                    gA = stq.tile([128, F4], f16, tag="gA")
                    nc.vector.tensor_mul(gA[:, 0:qw], qd["embB"][:, 0:qw],
                                         qd["wm0"][:, 0:qw])
                    qd["gA"] = gA

                def stage_gb(q):
                    qd = qstate[q]
                    qw = qd["qw"]
                    gB = stq.tile([128, F4], f16, tag="gB")
                    nc.vector.tensor_mul(gB[:, 0:qw], qd["embB"][:, 0:qw],
                                         qd["wm1"][:, 0:qw])
                    qd["gB"] = gB
                    # w1e for the whole quad: rows 16*jj+i = emb * w1, tile jj
                    qn = qw // F
                    w1e = stq.tile([64, F], f16, tag="w1e")
                    nc.vector.tensor_mul(w1e[0:16 * qn], qd["embB4"][0:16 * qn],
                                         qd["w1b4"][0:16 * qn])
                    qd["w1e"] = w1e

                def stage_gc(q):
                    qd = qstate[q]
                    qw = qd["qw"]
                    nc.vector.tensor_mul(qd["g2q"][0:112, 0:qw],
                                         qd["embB"][0:112, 0:qw],
                                         qd["wm2"][0:112, 0:qw])

                def stage_mains(t):
                    qd = qstate[t // 4]
                    j = t % 4
                    P = pP.tile([128, 2 * F], f32, tag="P")
                    rhs = (qd["gA"], qd["gB"], qd["g2q"])
                    for h in range(2):
                        for m in range(3):
                            nc.tensor.matmul(
                                P[:, F * h:F * (h + 1)],
                                lhsT=u3s[h][m][:KM[m]],
                                rhs=rhs[m][0:KM[m], F * j:F * (j + 1)],
                                start=(m == 0), stop=(m == 2),
                            )
                    state[t] = {"P": P}

                def stage_pc(t):
                    sd = state[t]
                    Pc = st.tile([128, 2 * F], f16, tag="Pc")
                    nc.scalar.copy(Pc[:], sd["P"][:])
                    sd["Pc"] = Pc

                def stage_s(t):
                    sd = state[t]
                    qd = qstate[t // 4]
                    j = t % 4
                    eB = qd["embB"][:, F * j:F * (j + 1)]
                    S = st.tile([128, 2 * F], f16, tag="S")
                    for h in range(2):
                        nc.vector.tensor_mul(S[:, F * h:F * (h + 1)],
                                             sd["Pc"][:, F * h:F * (h + 1)],
                                             eB)
                    sd["S"] = S

                def stage_ysel(t):
                    sd = state[t]
                    p1 = pP1.tile([16, F], f32, tag="p1", name="p1")
                    for h in range(2):
                        nc.tensor.matmul(p1[:], lhsT=sel[h][:],
                                         rhs=sd["S"][:, F * h:F * (h + 1)],
                                         start=(h == 0), stop=(h == 1))
                    sd["p1"] = p1

                def stage_s2(t):
                    sd = state[t]
                    qd = qstate[t // 4]
                    j = t % 4
                    j2 = j % 2
                    j2q = j // 2
                    if j2 == 0:
                        s2p = st.tile([96, F], f16, tag="s2p")
                        if t < 16:
                            nc.gpsimd.memset(s2p[:], 0.0)
                        state[t]["s2p"] = s2p
                        # both tiles' w1e rows, one copy from the quad tile
                        nr2 = 32 if t + 1 < nt else 16
                        nc.vector.tensor_copy(
                            s2p[64:64 + nr2],
                            qd["w1e"][32 * j2q:32 * j2q + nr2])
                    else:
                        s2p = state[t - 1]["s2p"]
                        state[t]["s2p"] = s2p
                    nc.vector.tensor_mul(s2p[32 * j2:32 * j2 + 16],
                                         sd["p1"][:],
                                         qd["embB"][0:16, F * j:F * (j + 1)])

                def stage_xred(t):
                    # one M=2 matmul per pair, fired at the odd tile
                    if t % 2 == 0 and t + 1 < nt:
                        return
                    sd = state[t]
                    p2 = pP2.tile([2, F], f32, tag="p2", name="p2")
                    nc.tensor.matmul(p2[:], lhsT=onesu1[:],
                                     rhs=sd["s2p"][:],
                                     start=True, stop=True)
                    sd["p2"] = p2

                def stage_o1(t):
                    if t % 2 == 0 and t + 1 < nt:
                        return
                    sd = state[t]
                    nr = t % 2 + 1
                    o1p = sto.tile([2, F], f32, tag="o1p")
                    nc.scalar.copy(o1p[0:nr], sd["p2"][0:nr])
                    sd["o1p"] = o1p

                def stage_qout(t):
                    # per-pair output store, fired at the odd tile (or tail)
                    if t % 2 == 0 and t + 1 < nt:
                        return
                    j2 = t % 2
                    nr = j2 + 1
                    node0p = TB * (t - j2)
                    sd = state[t]
                    nc.sync.dma_start(
                        out=out_d[node0p:node0p + TB * nr, :],
                        in_=sd["o1p"][0:nr],
                    )
                    state.pop(t, None)
                    state.pop(t - 1, None)

                def guard(f, t):
                    if 0 <= t < nt:
                        f(t)

                def qguard(f, q):
                    if 0 <= q < nq:
                        f(q)

                for u in range(nt + 14):
                    guard(stage_ysel, u - 9)
                    guard(stage_xred, u - 11)
                    if u % 4 == 0:
                        qguard(stage_load, u // 4)
                    elif u % 4 == 1:
                        qguard(stage_ga, u // 4)
                    elif u % 4 == 2:
                        qguard(stage_gb, u // 4)
                    else:
                        qguard(stage_gc, u // 4)
                    guard(stage_mains, u - 6)
                    guard(stage_pc, u - 7)
                    guard(stage_s, u - 8)
                    guard(stage_s2, u - 10)
                    guard(stage_o1, u - 12)
                    guard(stage_qout, u - 13)
    nc.compile()
    return nc


# ---------------- host-side input preparation ----------------

def _prep_constants(U3, U2, U1):
    """Stationary operands: U3/U2 reordered to (k-major ik rows, (x,y) cols)."""
    U3 = np.asarray(U3, dtype=np.float32)
    U2 = np.asarray(U2, dtype=np.float32)
    U1 = np.asarray(U1, dtype=np.float32)
    # rows r=(k,i)=k*16+i, cols (x,y)=x*16+y
    U3r = U3[0].transpose(3, 2, 0, 1).reshape(Z3 * Y, Y * Y)
    U2r = U2[0].transpose(2, 0, 1).reshape(Z2, Y * Y)
    M = np.vstack([U3r, U2r])                       # (372, 256)
    u3s = np.zeros((2, 3, 128, 128), dtype=np.float16)
    for m in range(3):
        chunk = M[128 * m:128 * m + KM[m]]
        for h in range(2):
            u3s[h, m, :KM[m], :] = chunk[:, 128 * h:128 * (h + 1)]
    sel = np.zeros((2, 128, 16), dtype=np.float16)
    for h in range(2):
        for p in range(128):
            sel[h, p, 8 * h + p // 16] = 1.0
    # pair-stacked xred weights [96, 2]: col j2 sums rows 32*j2+[0:16]
    # (ones, s2a) and 64 + 16*j2 + [0:16] (U1, w1e)
    onesu1 = np.zeros((96, 2), dtype=np.float16)
    for j2 in range(2):
        onesu1[32 * j2:32 * j2 + Y, j2] = 1.0
        onesu1[64 + Y * j2:64 + Y * (j2 + 1), j2] = U1[0, :, 0]
    return u3s, sel, onesu1


def _prep_core_inputs(emb_pad, attr_pad, wcat, consts, g, nb=NB):
    u3s, sel, onesu1 = consts
    sl = slice(g * nb, (g + 1) * nb)
    embT = np.ascontiguousarray(
        emb_pad[sl].transpose(2, 0, 1).reshape(Y, nb * C)
    ).astype(np.float16)
    attrT = np.ascontiguousarray(attr_pad[sl].T).astype(np.float16)
    return {
        "embT": embT,
        "attrT": attrT,
        "wcat": wcat,
        "u3s": u3s,
        "sel": sel,
        "onesu1": onesu1,
    }


def _prep_all(node_embeddings, node_attributes, U3, U2, U1, W3, W2, W1):
    emb = np.asarray(node_embeddings, dtype=np.float32)
    attr = np.asarray(node_attributes, dtype=np.float32)
    emb_pad = np.zeros((NPAD, C, Y), dtype=np.float32)
    emb_pad[:N] = emb
    attr_pad = np.zeros((NPAD, E), dtype=np.float32)
    attr_pad[:N] = attr
    # wcat[e, kk*C + c]: kk' 0..22 = W3, 23 = W1, 24..27 = W2
    wcat = np.concatenate(
        [np.asarray(W3, np.float32), np.asarray(W1, np.float32),
         np.asarray(W2, np.float32)], axis=1
    ).reshape(E, WROW).astype(np.float16)
    consts = _prep_constants(U3, U2, U1)
    return [
        _prep_core_inputs(emb_pad, attr_pad, wcat, consts, g)
        for g in range(NCORES)
    ]


def kernel(node_embeddings, node_attributes, U3, U2, U1, W3, W2, W1):
    from concourse.bass_utils import run_bass_kernel_spmd

    if "nc" not in _CACHE:
        _CACHE["nc"] = _build_program(NB)
    nc = _CACHE["nc"]
    in_maps = _prep_all(node_embeddings, node_attributes,
                        U3, U2, U1, W3, W2, W1)
    trace = bool(int(os.environ.get("KERNEL_TRACE", "0")))
    res = run_bass_kernel_spmd(
        nc, in_maps, core_ids=list(range(NCORES)), trace=trace,
    )
    _CACHE["last_results"] = res
    out = np.concatenate([res.results[g]["out"] for g in range(NCORES)], axis=0)
    return np.ascontiguousarray(out[:N]).astype(np.float32)


# revision 26
# speedup vs baseline: 1.4292x; 1.0091x over previous
"""Trainium2 Bass kernel for the MACE-style symmetric contraction.

Math (per node b, feature c, with emb = node_embeddings[b, c, :] (16,)):
    w{3,2,1}[k, c] = sum_e attr[b, e] * W{3,2,1}[e, k, c]
    out3[x, y] = sum_{i,k} emb[i] * w3[k] * U3[0, x, y, i, k]        (16, 16)
    M3[x, y]   = out3[x, y] + sum_k2 U2[0, x, y, k2] * w2[k2]
    o2[x]      = sum_y M3[x, y] * emb[y] + U1[0, x, 0] * w1[0]
    o1         = sum_x o2[x] * emb[x]
    output[b, c] = o1

Mapping: columns = (node-in-tile, c) pairs, 4 nodes x 128 c = 512 cols/tile.
The (i, k) contraction (k-major, 368 rows + 4 U2 rows) runs on the PE as
3 accumulating matmuls per output half (x,y) -> 256 rows in 2 halves of 128.

v2 layout: all per-tile weight replication arrives in ONE DMA (wmall3,
[128, 1536]), G-build runs on gpsimd as one op, ACT evacuates PSUM, DVE
does SBUF muls; p1/p2 PSUM banks are shared 4 tiles wide via column-strip
tile positions; outputs drain once per quad.
"""

import os

import numpy as np

# ---------------- problem constants (hardcoded per contract) ----------------
N, C, Y, E = 3000, 128, 16, 10
Z3, Z2, Z1 = 23, 4, 1
NCORES = 8
NB = 376                # nodes per core (3008 = 8*376, padded)
NPAD = NCORES * NB
TB = 4                  # nodes per tile
F = TB * C              # 512 columns per tile
NT = NB // TB           # 94 tiles
KK = Z3 + Z2 + Z1       # 28 packed k rows in wflat (kk' = [W3 0..22, W1 23, W2 24..27])
WROW = KK * C           # 3584: wflat row length
KM = (128, 128, 116)    # contraction chunk K sizes (368 ik rows + 4 U2 rows)

_CACHE = {}


def _build_program(nb):
    """Build the single-core Bass program (SPMD: same program, all cores)."""
    import concourse.bass as bass
    import concourse.mybir as mybir
    import concourse.tile as tile
    from concourse import bacc

    f16, f32 = mybir.dt.float16, mybir.dt.float32
    nt = nb // TB
    nc = bacc.Bacc(None, target_bir_lowering=False)

    embT_d = nc.dram_tensor("embT", [Y, nb * C], f16, kind="ExternalInput")
    attrT_d = nc.dram_tensor("attrT", [E, nb], f16, kind="ExternalInput")
    wcat_d = nc.dram_tensor("wcat", [E, WROW], f16, kind="ExternalInput")
    u3s_d = nc.dram_tensor("u3s", [2, 3, 128, 128], f16, kind="ExternalInput")
    sel_d = nc.dram_tensor("sel", [2, 128, 16], f16, kind="ExternalInput")
    onesu1_d = nc.dram_tensor("onesu1", [96, 2], f16, kind="ExternalInput")
    out_d = nc.dram_tensor("out", [nb, C], f32, kind="ExternalOutput")

    nbC = nb * C

    with tile.TileContext(nc) as tc:
        with tc.tile_pool(name="consts", bufs=1) as consts, \
             tc.tile_pool(name="dram", bufs=1, space="DRAM") as dpool:
            # stationaries, loaded once
            u3s = []
            for h in range(2):
                row = []
                for m in range(3):
                    t = consts.tile([128, 128], f16, tag=f"u3s{h}{m}")
                    nc.sync.dma_start(out=t[:], in_=u3s_d[h, m])
                    row.append(t)
                u3s.append(row)
            sel = []
            for h in range(2):
                t = consts.tile([128, 16], f16, tag=f"sel{h}")
                nc.sync.dma_start(out=t[:], in_=sel_d[h])
                sel.append(t)
            onesu1 = consts.tile([96, 2], f16, tag="onesu1")
            nc.sync.dma_start(out=onesu1[:], in_=onesu1_d[:])

            # PE warm-up: dependency-free matmuls push the HAM activity
            # window to K=8/8 (2.4 GHz) before real work starts.
            wuburst = consts.tile([128, 512], f16, tag="wuburst")
            nc.gpsimd.memset(wuburst[:], 0.0)
            with tc.tile_pool(name="psW", bufs=1, space="PSUM") as psW:
                wups = psW.tile([128, 512], f32, tag="wups")
                for _ in range(30):
                    nc.tensor.matmul(wups[:], lhsT=u3s[0][0][:], rhs=wuburst[:],
                                     start=True, stop=True)

            # wflatT[kk, node*C + c] = sum_e attr[node, e] * Wcat[e, kk*C + c]
            wflatT = dpool.tile([KK, nbC], f16, tag="wflatT")

            # ---------------- phase A: produce wflatT ----------------
            with tc.tile_pool(name="pa", bufs=4) as pa, \
                 tc.tile_pool(name="psA", bufs=4, space="PSUM") as psA:
                attrT = pa.tile([E, nb], f16, tag="attrT")
                nc.sync.dma_start(out=attrT[:], in_=attrT_d[:])
                wcat = pa.tile([E, WROW], f16, tag="wcat")
                nc.sync.dma_start(out=wcat[:], in_=wcat_d[:])
                wflatT_ap = wflatT[:]
                for gs in range(0, nb, 128):
                    gn = min(128, nb - gs)
                    for j in range(WROW // 512):
                        pw = psA.tile([128, 512], f32, tag="pw")
                        nc.tensor.matmul(
                            pw[:gn],
                            lhsT=attrT[:, gs:gs + gn],
                            rhs=wcat[:, 512 * j:512 * (j + 1)],
                            start=True, stop=True,
                        )
                        wf = pa.tile([128, 512], f16, tag="wf")
                        nc.vector.tensor_copy(wf[:gn], pw[:gn])
                        # scatter-transpose: (node, 4 kk, c) -> wflatT rows
                        nc.gpsimd.dma_start(
                            out=bass.AP(
                                tensor=wflatT_ap.tensor,
                                offset=wflatT_ap.offset + 4 * j * nbC + gs * C,
                                ap=[[C, gn], [nbC, 4], [1, C]],
                            ),
                            in_=wf[:gn],
                        )

            # ---------------- phase B: main loop ----------------
            wflatT_ap = wflatT[:]
            embT_ap = embT_d[:]
            F4 = 4 * F
            nq = (nt + 3) // 4

            # Quad-granular loads and G ops (big ops amortize fixed costs),
            # per-tile matmul pipeline, quad-granular output drain.
            with tc.tile_pool(name="stq", bufs=5) as stq, \
                 tc.tile_pool(name="st", bufs=8) as st, \
                 tc.tile_pool(name="sto", bufs=2) as sto, \
                 tc.tile_pool(name="pP", bufs=2, space="PSUM") as pP, \
                 tc.tile_pool(name="pP1", bufs=2, space="PSUM") as pP1, \
                 tc.tile_pool(name="pP2", bufs=2, space="PSUM") as pP2:
                qstate = {}
                state = {}

                def stage_load(q):
                    col0 = 4 * q * F
                    qw = min(F4, nbC - col0)
                    # embB[p, f] = emb[p % 16, col0 + f]  (8x partition rep)
                    embB = stq.tile([128, F4], f16, tag="embB")
                    nc.sync.dma_start(
                        out=embB[:, 0:qw],
                        in_=bass.AP(
                            tensor=embT_ap.tensor,
                            offset=embT_ap.offset + col0,
                            ap=[[0, 8], [nbC, Y], [1, qw]],
                        ),
                    )
                    # wm{m}[p = kp*16+i, f] = wflat[8m + kp, col0 + f]
                    def wmdma(eng, kk0, rows):
                        wm = stq.tile([rows, F4], f16, tag=f"wm{kk0}",
                                      name=f"wm{kk0}")
                        eng.dma_start(
                            out=wm[:, 0:qw],
                            in_=bass.AP(
                                tensor=wflatT_ap.tensor,
                                offset=wflatT_ap.offset + kk0 * nbC + col0,
                                ap=[[nbC, rows // 16], [0, 16], [1, qw]],
                            ),
                        )
                        return wm
                    wm0 = wmdma(nc.sync, 0, 128)
                    wm1 = wmdma(nc.sync, 8, 128)
                    wm2 = wmdma(nc.sync, 16, 128)
                    qn = qw // F
                    # embB4[16*jj + i, f] = emb[i, col0 + F*jj + f]
                    embB4 = stq.tile([64, F], f16, tag="embB4")
                    nc.scalar.dma_start(
                        out=embB4[0:16 * qn],
                        in_=bass.AP(
                            tensor=embT_ap.tensor,
                            offset=embT_ap.offset + col0,
                            ap=[[F, qn], [nbC, 16], [1, F]],
                        ),
                    )
                    # w1b4[16*jj + i, f] = wflat[23, col0 + F*jj + f]
                    w1b4 = stq.tile([64, F], f16, tag="w1b4")
                    nc.scalar.dma_start(
                        out=w1b4[0:16 * qn],
                        in_=bass.AP(
                            tensor=wflatT_ap.tensor,
                            offset=wflatT_ap.offset + 23 * nbC + col0,
                            ap=[[F, qn], [0, 16], [1, F]],
                        ),
                    )
                    # U2 rows (kk' 24..27) direct into g2q[112:116]
                    g2q = stq.tile([116, F4], f16, tag="g2q")
                    nc.scalar.dma_start(
                        out=g2q[112:116, 0:qw],
                        in_=bass.AP(
                            tensor=wflatT_ap.tensor,
                            offset=wflatT_ap.offset + 24 * nbC + col0,
                            ap=[[nbC, 4], [1, qw]],
                        ),
                    )
                    qstate[q] = {"embB": embB, "wm0": wm0, "wm1": wm1,
                                 "wm2": wm2, "embB4": embB4, "w1b4": w1b4,
                                 "g2q": g2q, "qw": qw}

                def stage_ga(q):
                    qd = qstate[q]
                    qw = qd["qw"]
                    gA = stq.tile([128, F4], f16, tag="gA")
                    nc.vector.tensor_mul(gA[:, 0:qw], qd["embB"][:, 0:qw],
                                         qd["wm0"][:, 0:qw])
                    qd["gA"] = gA

                def stage_gb(q):
                    qd = qstate[q]
                    qw = qd["qw"]
                    gB = stq.tile([128, F4], f16, tag="gB")
                    nc.vector.tensor_mul(gB[:, 0:qw], qd["embB"][:, 0:qw],
                                         qd["wm1"][:, 0:qw])
                    qd["gB"] = gB
                    # w1e for the whole quad: rows 16*jj+i = emb * w1, tile jj
                    qn = qw // F
                    w1e = stq.tile([64, F], f16, tag="w1e")
                    nc.vector.tensor_mul(w1e[0:16 * qn], qd["embB4"][0:16 * qn],
                                         qd["w1b4"][0:16 * qn])
                    qd["w1e"] = w1e

                def stage_gc(q):
                    qd = qstate[q]
                    qw = qd["qw"]
                    nc.vector.tensor_mul(qd["g2q"][0:112, 0:qw],
                                         qd["embB"][0:112, 0:qw],
                                         qd["wm2"][0:112, 0:qw])

                def stage_mains(t):
                    qd = qstate[t // 4]
                    j = t % 4
                    P = pP.tile([128, 2 * F], f32, tag="P")
                    rhs = (qd["gA"], qd["gB"], qd["g2q"])
                    for h in range(2):
                        for m in range(3):
                            nc.tensor.matmul(
                                P[:, F * h:F * (h + 1)],
                                lhsT=u3s[h][m][:KM[m]],
                                rhs=rhs[m][0:KM[m], F * j:F * (j + 1)],
                                start=(m == 0), stop=(m == 2),
                            )
                    state[t] = {"P": P}

                def stage_pc(t):
                    sd = state[t]
                    Pc = st.tile([128, 2 * F], f16, tag="Pc")
                    nc.scalar.copy(Pc[:], sd["P"][:])
                    sd["Pc"] = Pc

                def stage_s(t):
                    sd = state[t]
                    qd = qstate[t // 4]
                    j = t % 4
                    eB = qd["embB"][:, F * j:F * (j + 1)]
                    S = st.tile([128, 2 * F], f16, tag="S")
                    for h in range(2):
                        nc.vector.tensor_mul(S[:, F * h:F * (h + 1)],
                                             sd["Pc"][:, F * h:F * (h + 1)],
                                             eB)
                    sd["S"] = S

                def stage_ysel(t):
                    sd = state[t]
                    p1 = pP1.tile([16, F], f32, tag="p1", name="p1")
                    for h in range(2):
                        nc.tensor.matmul(p1[:], lhsT=sel[h][:],
                                         rhs=sd["S"][:, F * h:F * (h + 1)],
                                         start=(h == 0), stop=(h == 1))
                    sd["p1"] = p1

                def stage_s2(t):
                    sd = state[t]
                    qd = qstate[t // 4]
                    j = t % 4
                    j2 = j % 2
                    j2q = j // 2
                    if j2 == 0:
                        s2p = st.tile([96, F], f16, tag="s2p")
                        if t < 16:
                            nc.gpsimd.memset(s2p[:], 0.0)
                        state[t]["s2p"] = s2p
                        # both tiles' w1e rows, one copy from the quad tile
                        nr2 = 32 if t + 1 < nt else 16
                        nc.vector.tensor_copy(
                            s2p[64:64 + nr2],
                            qd["w1e"][32 * j2q:32 * j2q + nr2])
                    else:
                        s2p = state[t - 1]["s2p"]
                        state[t]["s2p"] = s2p
                    nc.vector.tensor_mul(s2p[32 * j2:32 * j2 + 16],
                                         sd["p1"][:],
                                         qd["embB"][0:16, F * j:F * (j + 1)])

                def stage_xred(t):
                    # one M=2 matmul per pair, fired at the odd tile
                    if t % 2 == 0 and t + 1 < nt:
                        return
                    sd = state[t]
                    p2 = pP2.tile([2, F], f32, tag="p2", name="p2")
                    nc.tensor.matmul(p2[:], lhsT=onesu1[:],
                                     rhs=sd["s2p"][:],
                                     start=True, stop=True)
                    sd["p2"] = p2

                def stage_o1(t):
                    if t % 2 == 0 and t + 1 < nt:
                        return
                    sd = state[t]
                    nr = t % 2 + 1
                    o1p = sto.tile([2, F], f32, tag="o1p")
                    nc.scalar.copy(o1p[0:nr], sd["p2"][0:nr])
                    sd["o1p"] = o1p

                def stage_qout(t):
                    # per-pair output store, fired at the odd tile (or tail)
                    if t % 2 == 0 and t + 1 < nt:
                        return
                    j2 = t % 2
                    nr = j2 + 1
                    node0p = TB * (t - j2)
                    sd = state[t]
                    nc.sync.dma_start(
                        out=out_d[node0p:node0p + TB * nr, :],
                        in_=sd["o1p"][0:nr],
                    )
                    state.pop(t, None)
                    state.pop(t - 1, None)

                def guard(f, t):
                    if 0 <= t < nt:
                        f(t)

                def qguard(f, q):
                    if 0 <= q < nq:
                        f(q)

                for u in range(nt + 17):
                    guard(stage_ysel, u - 10)
                    guard(stage_xred, u - 12)
                    if u % 4 == 0:
                        qguard(stage_load, u // 4)
                    elif u % 4 == 1:
                        qguard(stage_ga, u // 4)
                    elif u % 4 == 2:
                        qguard(stage_gb, u // 4)
                    else:
                        qguard(stage_gc, u // 4)
                    guard(stage_mains, u - 6)
                    guard(stage_pc, u - 7)
                    guard(stage_s, u - 8)
                    guard(stage_s2, u - 11)
                    guard(stage_o1, u - 13)
                    guard(stage_qout, u - 14)
    nc.compile()
    return nc


# ---------------- host-side input preparation ----------------

def _prep_constants(U3, U2, U1):
    """Stationary operands: U3/U2 reordered to (k-major ik rows, (x,y) cols)."""
    U3 = np.asarray(U3, dtype=np.float32)
    U2 = np.asarray(U2, dtype=np.float32)
    U1 = np.asarray(U1, dtype=np.float32)
    # rows r=(k,i)=k*16+i, cols (x,y)=x*16+y
    U3r = U3[0].transpose(3, 2, 0, 1).reshape(Z3 * Y, Y * Y)
    U2r = U2[0].transpose(2, 0, 1).reshape(Z2, Y * Y)
    M = np.vstack([U3r, U2r])                       # (372, 256)
    u3s = np.zeros((2, 3, 128, 128), dtype=np.float16)
    for m in range(3):
        chunk = M[128 * m:128 * m + KM[m]]
        for h in range(2):
            u3s[h, m, :KM[m], :] = chunk[:, 128 * h:128 * (h + 1)]
    sel = np.zeros((2, 128, 16), dtype=np.float16)
    for h in range(2):
        for p in range(128):
            sel[h, p, 8 * h + p // 16] = 1.0
    # pair-stacked xred weights [96, 2]: col j2 sums rows 32*j2+[0:16]
    # (ones, s2a) and 64 + 16*j2 + [0:16] (U1, w1e)
    onesu1 = np.zeros((96, 2), dtype=np.float16)
    for j2 in range(2):
        onesu1[32 * j2:32 * j2 + Y, j2] = 1.0
        onesu1[64 + Y * j2:64 + Y * (j2 + 1), j2] = U1[0, :, 0]
    return u3s, sel, onesu1


def _prep_core_inputs(emb_pad, attr_pad, wcat, consts, g, nb=NB):
    u3s, sel, onesu1 = consts
    sl = slice(g * nb, (g + 1) * nb)
    embT = np.ascontiguousarray(
        emb_pad[sl].transpose(2, 0, 1).reshape(Y, nb * C)
    ).astype(np.float16)
    attrT = np.ascontiguousarray(attr_pad[sl].T).astype(np.float16)
    return {
        "embT": embT,
        "attrT": attrT,
        "wcat": wcat,
        "u3s": u3s,
        "sel": sel,
        "onesu1": onesu1,
    }


def _prep_all(node_embeddings, node_attributes, U3, U2, U1, W3, W2, W1):
    emb = np.asarray(node_embeddings, dtype=np.float32)
    attr = np.asarray(node_attributes, dtype=np.float32)
    emb_pad = np.zeros((NPAD, C, Y), dtype=np.float32)
    emb_pad[:N] = emb
    attr_pad = np.zeros((NPAD, E), dtype=np.float32)
    attr_pad[:N] = attr
    # wcat[e, kk*C + c]: kk' 0..22 = W3, 23 = W1, 24..27 = W2
    wcat = np.concatenate(
        [np.asarray(W3, np.float32), np.asarray(W1, np.float32),
         np.asarray(W2, np.float32)], axis=1
    ).reshape(E, WROW).astype(np.float16)
    consts = _prep_constants(U3, U2, U1)
    return [
        _prep_core_inputs(emb_pad, attr_pad, wcat, consts, g)
        for g in range(NCORES)
    ]


def kernel(node_embeddings, node_attributes, U3, U2, U1, W3, W2, W1):
    from concourse.bass_utils import run_bass_kernel_spmd

    if "nc" not in _CACHE:
        _CACHE["nc"] = _build_program(NB)
    nc = _CACHE["nc"]
    in_maps = _prep_all(node_embeddings, node_attributes,
                        U3, U2, U1, W3, W2, W1)
    trace = bool(int(os.environ.get("KERNEL_TRACE", "0")))
    res = run_bass_kernel_spmd(
        nc, in_maps, core_ids=list(range(NCORES)), trace=trace,
    )
    _CACHE["last_results"] = res
    out = np.concatenate([res.results[g]["out"] for g in range(NCORES)], axis=0)
    return np.ascontiguousarray(out[:N]).astype(np.float32)
